# revision 14
# baseline (speedup 1.0000x reference)
"""Trainium2 Bass kernel for Chn8ActGrp3WgtQuantizedLinear.

Computes: out = fake_quant8_per_row(x) @ dequant(weight_qvals, weight_scales).T

  x:             (1024, 4096)  f32
  weight_qvals:  (11008, 4096) int32, 3-bit values in [-4, 3]
  weight_scales: (11008, 32)   f32, one scale per (out-channel, 128-group)
  out:           (1024, 11008) f32
  group_size:    128

Strategy (tensor parallel over 8 NeuronCores; N=11008 -> 1376/core):
  - host repack (layout/dtype only): x -> fp16; wq -> K-major fp16
    [4096, 1376] (3-bit values exact in fp16); ws -> fp16 compact
    [128, 2752]: per k-group-pair block b, partitions 8b..8b+7 hold
    concat(ws[:,2b], ws[:,2b+1]) (0.7 MB vs 11.3 MB full broadcast).
  - device per core:
      * ws block broadcast 8 -> 128 partitions with ONE stride-0-source
        SBUF->SBUF DMA per block (gpsimd queue); dequant
        W[k,n] = wq * ws_bc on DVE (fp16 2x mode).
      * activation fake-quant per 128-row m-tile: row min/max via a
        tensor_tensor halving tree (fp16 2x) + one 1x reduce on DVE;
        scale/inv; then a = round(x*inv) with the fp16 magic trick
        (u = ACT(x*inv + 1536) -> f16 rounds at the output convert since
        ulp(1536)=1 for |v|<512; a = u - 1536 on DVE, all exact).
        The clip to [qmin-z, qmax-z] is dropped: without clipping the
        zero-point cancels algebraically (a = round(x/s)), and round(x/s)
        escapes the clip range only by 1 lsb on knife-edge row extremes,
        perturbing a handful of elements by one quant step.
      * aT[k, m] via DMA-xbar transposes (two halves per m-tile:
        a[128m, 2048] -> aT[128k', 16g, 128m]); PE runs matmuls only.
      * matmul: psum[m=128, n=1376] += aT[:,g,:].T @ W over 32 k-groups
        (512-col psum-bank chunks); m0/m1 group-interleaved so the W
        DMA/dequant ramp is consumed at 2 m-tiles per group.
      * evict with per-row scale: out = psum * scale[m] (ACT), DMA out.
  - host concatenates the 8 (1024, 1376) shards.
"""

import sys
import types

import numpy as np

M, K, N, GS = 1024, 4096, 11008, 128
NCORES = 8
NC_SHARD = N // NCORES  # 1376
NGRP = K // GS  # 32
NBLK = NGRP // 2  # 16 k-group-pair blocks for the ws stream
MTILES = M // 128  # 8
MAGIC = 1536.0  # 1.5 * 2**10: f16 output convert rounds x*inv to int (RNE)
WS_REP = 8  # ws host replication rows per block

_CACHE = {}
LAST_RESULTS = None


def _install_axon_ntff_hook():
    """Register the NTFF profile hook if the container's antenv lacks it.

    Only needed for trace=True (BASS_TRACE=1); degrades silently."""
    try:
        if "antenv.axon_hooks" in sys.modules:
            return
        import antenv

        mod = types.ModuleType("antenv.axon_hooks")
        _state = {"hook": None}
        mod.set_axon_ntff_profile_hook = lambda h: _state.__setitem__("hook", h)
        mod.get_axon_ntff_profile_hook = lambda: _state["hook"]
        sys.modules["antenv.axon_hooks"] = mod
        antenv.axon_hooks = mod

        from trn_agent_boot.trn_boot import _ntff_profile_via_ctypes

        mod.set_axon_ntff_profile_hook(
            _ntff_profile_via_ctypes("/opt/axon/libaxon_pjrt.so")
        )
    except Exception:
        pass


def _build():
    if "nc" in _CACHE:
        return _CACHE["nc"]

    import contextlib

    import concourse.tile as tile
    from concourse import bacc, mybir

    dt = mybir.dt
    F32, F16 = dt.float32, dt.float16
    ALU = mybir.AluOpType
    ACTF = mybir.ActivationFunctionType
    AX = mybir.AxisListType

    nc = bacc.Bacc("TRN2", target_bir_lowering=False, debug=False,
                   num_devices=NCORES)

    x_d = nc.dram_tensor("x", [M, K], F16, kind="ExternalInput").ap()
    wq_d = nc.dram_tensor("wq", [K, NC_SHARD], F16, kind="ExternalInput").ap()
    ws_d = nc.dram_tensor("ws", [NBLK * WS_REP, 2 * NC_SHARD], F16,
                          kind="ExternalInput").ap()
    out_d = nc.dram_tensor("out", [M, NC_SHARD], F32, kind="ExternalOutput").ap()

    CHUNKS = [(c, min(512, NC_SHARD - c)) for c in range(0, NC_SHARD, 512)]

    with tile.TileContext(nc) as tc:
        ctx = contextlib.ExitStack()
        with ctx:
            consts = ctx.enter_context(tc.tile_pool(name="consts", bufs=1))
            wpool = ctx.enter_context(tc.tile_pool(name="w", bufs=1))
            wqld = ctx.enter_context(tc.tile_pool(name="wqld", bufs=2))
            ws8p = ctx.enter_context(tc.tile_pool(name="ws8", bufs=1))
            wsb = ctx.enter_context(tc.tile_pool(name="ws", bufs=2))
            xp = ctx.enter_context(tc.tile_pool(name="x", bufs=3))
            up = ctx.enter_context(tc.tile_pool(name="u", bufs=2))
            ap_ = ctx.enter_context(tc.tile_pool(name="a", bufs=2))
            atp = ctx.enter_context(tc.tile_pool(name="at", bufs=3))
            tre = ctx.enter_context(tc.tile_pool(name="tree", bufs=1))
            outp = ctx.enter_context(tc.tile_pool(name="o", bufs=1))
            vecs = ctx.enter_context(tc.tile_pool(name="v", bufs=8))
            ps_out = ctx.enter_context(
                tc.tile_pool(name="pso", bufs=2, space="PSUM"))

            magic_vec = consts.tile([128, 1], F32)
            nc.vector.memset(magic_vec[:], MAGIC)

            # W holds all dequantized weights, k-major: [k%128, g, n]
            W = wpool.tile([128, NGRP * NC_SHARD], F16)
            ws8 = ws8p.tile([128, 2 * NC_SHARD], F16)

            x_of = {}
            scale_of = {}
            inv_of = {}
            a_of = {}
            at_of = {}

            def load_x(m, chunked=False):
                x_t = xp.tile([128, K], F16, tag="xt")
                if chunked:  # startup: stats can begin before full row lands
                    for j in range(4):
                        sl = slice(j * 1024, (j + 1) * 1024)
                        nc.scalar.dma_start(x_t[:, sl],
                                            x_d[m * 128:(m + 1) * 128, sl])
                else:
                    nc.scalar.dma_start(x_t[:], x_d[m * 128:(m + 1) * 128, :])
                x_of[m] = x_t

            def stats(m, chunked=False):
                """DVE row min/max via fp16 tensor_tensor tree + 1x reduce."""
                x_t = x_of[m]
                mx = vecs.tile([128, 1], F32, tag="mx")
                mn = vecs.tile([128, 1], F32, tag="mn")
                if chunked:  # per-1024 chunk partials, gated on each x DMA
                    mxp = vecs.tile([128, 4], F32, tag="mxp")
                    mnp = vecs.tile([128, 4], F32, tag="mnp")
                    for j in range(4):
                        lo = slice(j * 1024, j * 1024 + 512)
                        hi = slice(j * 1024 + 512, (j + 1) * 1024)
                        for (op, prt) in ((ALU.max, mxp), (ALU.min, mnp)):
                            s2 = tre.tile([128, 512], F16, tag="s2c")
                            nc.vector.tensor_tensor(s2[:], x_t[:, lo],
                                                    x_t[:, hi], op)
                            nc.vector.tensor_reduce(prt[:, j:j + 1], s2[:],
                                                    axis=AX.X, op=op)
                    nc.vector.tensor_reduce(mx[:], mxp[:], axis=AX.X, op=ALU.max)
                    nc.vector.tensor_reduce(mn[:], mnp[:], axis=AX.X, op=ALU.min)
                else:
                    for (op, dst) in ((ALU.max, mx), (ALU.min, mn)):
                        s1 = tre.tile([128, 2048], F16, tag="s1")
                        nc.vector.tensor_tensor(s1[:], x_t[:, :2048],
                                                x_t[:, 2048:], op)
                        s2 = tre.tile([128, 1024], F16, tag="s2")
                        nc.vector.tensor_tensor(s2[:], s1[:, :1024],
                                                s1[:, 1024:], op)
                        nc.vector.tensor_reduce(dst[:], s2[:], axis=AX.X, op=op)
                xc = vecs.tile([128, 1], F32, tag="xc")
                nc.vector.tensor_scalar(xc[:], mx[:], 0.0, None, ALU.max)
                nn_ = vecs.tile([128, 1], F32, tag="nn")
                nc.vector.tensor_scalar(nn_[:], mn[:], 0.0, None, ALU.min)
                df = vecs.tile([128, 1], F32, tag="df")
                nc.vector.tensor_tensor(df[:], xc[:], nn_[:], ALU.subtract)
                sc = vecs.tile([128, 1], F32, tag="sc")
                nc.vector.tensor_scalar(sc[:], df[:], 1.0 / 255.0, 1e-9,
                                        ALU.mult, ALU.max)
                inv = vecs.tile([128, 1], F32, tag="inv")
                nc.vector.reciprocal(inv[:], sc[:])
                scale_of[m] = sc
                inv_of[m] = inv

            def u_pass(m):
                """ACT: u = f16(x*inv + 1536) — the f16 convert rounds (RNE)."""
                u = up.tile([128, K], F16, tag="u")
                nc.scalar.activation(u[:], x_of[m][:], ACTF.Identity,
                                     bias=magic_vec[:], scale=inv_of[m][:])
                a_of[m] = ("u", u)

            def a_pass(m):
                """DVE (2x): a = u - 1536 (exact integers in fp16)."""
                u = a_of[m][1]
                a_t = ap_.tile([128, K], F16, tag="a")
                nc.vector.tensor_scalar(a_t[:], u[:], -MAGIC, None, ALU.add)
                a_of[m] = ("a", a_t)

            def trT(m, eng):
                """Two DMA-xbar transposes: a[128m, 2048] -> aT[128k',16g,128m]."""
                a_t = a_of[m][1]
                aT = atp.tile([128, NGRP, 128], F16, tag="aT")
                h = NGRP // 2
                eng.dma_start_transpose(aT[:, 0:h, :], a_t[:, :K // 2])
                eng.dma_start_transpose(aT[:, h:NGRP, :], a_t[:, K // 2:])
                at_of[m] = aT

            def ws_bcast(b):
                """One stride-0-source DMA: rows 8b..8b+7 -> 128 partitions."""
                rep = 128 // WS_REP
                ws_bc = wsb.tile([128, 2 * NC_SHARD], F16, tag="wsb")
                nc.gpsimd.dma_start(
                    ws_bc[:],
                    ws8[b * WS_REP:(b + 1) * WS_REP, :].unsqueeze(1)
                    .to_broadcast([WS_REP, rep, 2 * NC_SHARD]))
                return ws_bc

            def wq_deq(g, ws_bc, j):
                wq_t = wqld.tile([128, NC_SHARD], F16, tag="wq")
                nc.sync.dma_start(wq_t[:], wq_d[g * 128:(g + 1) * 128, :])
                nc.vector.tensor_tensor(
                    W[:, g * NC_SHARD:(g + 1) * NC_SHARD], wq_t[:],
                    ws_bc[:, j * NC_SHARD:(j + 1) * NC_SHARD], ALU.mult)

            def mm_group(psum, aT, g):
                for (c0, cw) in CHUNKS:
                    nc.tensor.matmul(psum[:, c0:c0 + cw],
                                     lhsT=aT[:, g, :],
                                     rhs=W[:, g * NC_SHARD + c0:
                                           g * NC_SHARD + c0 + cw],
                                     start=(g == 0), stop=(g == NGRP - 1))

            def evict(m, psum):
                o_t = outp.tile([128, NC_SHARD], F32, tag="o")
                nc.scalar.activation(o_t[:], psum[:], ACTF.Identity,
                                     bias=0.0, scale=scale_of[m][:])
                nc.sync.dma_start(out_d[m * 128:(m + 1) * 128, :], o_t[:])

            # ---- emission ----
            nc.sync.dma_start(ws8[:], ws_d[:, :])
            load_x(0, chunked=True)
            load_x(1)
            load_x(2)
            stats(0, chunked=True)
            u_pass(0)
            stats(1)
            u_pass(1)

            # W stream (2 wq groups + dequant per ws block), with m0/m1 a/trT
            # and m2/m3 quant interleaved so no engine queue blocks another.
            for b in range(NBLK):
                ws_bc = ws_bcast(b)
                for j in range(2):
                    wq_deq(2 * b + j, ws_bc, j)
                if b == 0:
                    a_pass(0)
                    trT(0, nc.scalar)
                if b == 1:
                    a_pass(1)
                    trT(1, nc.scalar)
                if b == 3:
                    load_x(3)
                    stats(2)
                    u_pass(2)
                if b == 7:
                    load_x(4)
                    stats(3)
                    u_pass(3)
                if b == 8:
                    a_pass(2)
                if b == 12:
                    a_pass(3)
            trT(2, nc.scalar)
            trT(3, nc.scalar)

            # fused m0+m1 matmul ramp: both consume each W group as it lands
            ps0 = ps_out.tile([128, NC_SHARD], F32, tag="psum")
            ps1 = ps_out.tile([128, NC_SHARD], F32, tag="psum")
            for g in range(NGRP):
                mm_group(ps0, at_of[0], g)
                mm_group(ps1, at_of[1], g)
            evict(0, ps0)
            evict(1, ps1)

            for m in range(2, MTILES):
                if m + 3 < MTILES:
                    load_x(m + 3)
                if m + 2 < MTILES:
                    stats(m + 2)
                    u_pass(m + 2)
                    a_pass(m + 2)
                    trT(m + 2, nc.scalar)
                psum = ps_out.tile([128, NC_SHARD], F32, tag="psum")
                for g in range(NGRP):
                    mm_group(psum, at_of[m], g)
                evict(m, psum)

    nc.compile()
    _CACHE["nc"] = nc
    return nc


def kernel(x, weight_qvals, weight_scales, group_size):
    global LAST_RESULTS
    _install_axon_ntff_hook()
    from concourse.bass_utils import run_bass_kernel_spmd

    x = np.asarray(x, dtype=np.float32)
    wq = np.asarray(weight_qvals)
    ws = np.asarray(weight_scales, dtype=np.float32)
    assert int(group_size) == GS
    assert x.shape == (M, K) and wq.shape == (N, K) and ws.shape == (N, NGRP)

    nc = _build()

    x16 = x.astype(np.float16)
    in_maps = []
    for c in range(NCORES):
        sl = slice(c * NC_SHARD, (c + 1) * NC_SHARD)
        wq_c = np.ascontiguousarray(wq[sl].T).astype(np.float16)
        # ws rows per block b: concat(ws[:,2b], ws[:,2b+1]), replicated x8
        ws_t = ws[sl].T.astype(np.float16)  # [32, 1376]
        ws_rows = ws_t.reshape(NBLK, 2 * NC_SHARD)
        ws_c = np.ascontiguousarray(
            np.broadcast_to(ws_rows[:, None, :], (NBLK, WS_REP, 2 * NC_SHARD))
        ).reshape(NBLK * WS_REP, 2 * NC_SHARD)
        in_maps.append({"x": x16, "wq": wq_c, "ws": ws_c})

    res = run_bass_kernel_spmd(nc, in_maps, core_ids=list(range(NCORES)))
    LAST_RESULTS = res
    out = np.concatenate([r["out"] for r in res.results], axis=1)
    return out


if __name__ == "__main__":
    rng = np.random.default_rng(0)
    xv = rng.standard_normal((M, K)).astype(np.float32)
    wqv = rng.integers(-4, 4, (N, K)).astype(np.int32)
    wsv = (rng.random((N, NGRP)).astype(np.float32) * 0.02 + 1e-4)
    o = kernel(xv, wqv, wsv, GS)
    print("out shape:", o.shape, "finite:", np.isfinite(o).all())


# revision 15
# speedup vs baseline: 1.4503x; 1.4503x over previous
"""Trainium2 Bass kernel for Chn8ActGrp3WgtQuantizedLinear.

Computes: out = fake_quant8_per_row(x) @ dequant(weight_qvals, weight_scales).T

  x:             (1024, 4096)  f32
  weight_qvals:  (11008, 4096) int32, 3-bit values in [-4, 3]
  weight_scales: (11008, 32)   f32, one scale per (out-channel, 128-group)
  out:           (1024, 11008) f32
  group_size:    128

Strategy (tensor parallel over 8 NeuronCores; N=11008 -> 1376/core):
  - host repack (layout/dtype only): x -> fp16; wq -> K-major fp16
    [4096, 1376] (3-bit values exact in fp16); ws -> fp16 pre-broadcast
    [16*128, 2752] (block b = groups 2b/2b+1 on 128 partitions).
  - device per core:
      * dequant W[k,n] = wq * ws_bc on DVE (fp16 2x mode), streamed per
        k-group as the wq/ws DMAs land.
      * activation fake-quant per 128-row m-tile: row min/max via a
        tensor_tensor halving tree (fp16 2x) + one 1x reduce on DVE;
        scale/inv; u = ACT(x*inv + 1536) -> f16 (the output convert
        rounds to integer, RNE, since ulp(1536)=1 for |v|<512); then
        in-place DVE u -= 1536 -> exact integer activations in fp16.
        The clip to [qmin-z, qmax-z] is dropped: without clipping the
        zero-point cancels algebraically (a = round(x/s)); round(x/s)
        escapes the clip range only by 1 lsb on knife-edge row extremes,
        perturbing a handful of elements by one quant step.
      * aT[k, m] via PE transposes (32 per m-tile) staged through fp16
        PSUM tiles (8 groups each) + ACT copies to SBUF.
      * matmul: psum[m=128, n=1376] += aT[:,g,:].T @ W over 32 k-groups
        (512-col psum-bank chunks); m0/m1 group-interleaved so the W
        DMA/dequant ramp is consumed at 2 m-tiles per group; quant for
        m2..m4 pipelined inside the ramp.
      * evict with per-row scale: out = psum * scale[m] (ACT), DMA out.
  - host concatenates the 8 (1024, 1376) shards.
"""

import sys
import types

import numpy as np

M, K, N, GS = 1024, 4096, 11008, 128
NCORES = 8
NC_SHARD = N // NCORES  # 1376
NGRP = K // GS  # 32
NBLK = NGRP // 2  # 16 k-group-pair blocks for the ws stream
MTILES = M // 128  # 8
MAGIC = 1536.0  # 1.5 * 2**10: f16 output convert rounds x*inv to int (RNE)

_CACHE = {}
LAST_RESULTS = None


def _install_axon_ntff_hook():
    """Register the NTFF profile hook if the container's antenv lacks it.

    Only needed for trace=True (BASS_TRACE=1); degrades silently."""
    try:
        if "antenv.axon_hooks" in sys.modules:
            return
        import antenv

        mod = types.ModuleType("antenv.axon_hooks")
        _state = {"hook": None}
        mod.set_axon_ntff_profile_hook = lambda h: _state.__setitem__("hook", h)
        mod.get_axon_ntff_profile_hook = lambda: _state["hook"]
        sys.modules["antenv.axon_hooks"] = mod
        antenv.axon_hooks = mod

        from trn_agent_boot.trn_boot import _ntff_profile_via_ctypes

        mod.set_axon_ntff_profile_hook(
            _ntff_profile_via_ctypes("/opt/axon/libaxon_pjrt.so")
        )
    except Exception:
        pass


def _build():
    if "nc" in _CACHE:
        return _CACHE["nc"]

    import contextlib

    import concourse.tile as tile
    from concourse import bacc, mybir
    from concourse.masks import make_identity

    dt = mybir.dt
    F32, F16 = dt.float32, dt.float16
    ALU = mybir.AluOpType
    ACTF = mybir.ActivationFunctionType
    AX = mybir.AxisListType

    nc = bacc.Bacc("TRN2", target_bir_lowering=False, debug=False,
                   num_devices=NCORES)

    x_d = nc.dram_tensor("x", [M, K], F16, kind="ExternalInput").ap()
    wq_d = nc.dram_tensor("wq", [K, NC_SHARD], F16, kind="ExternalInput").ap()
    ws_d = nc.dram_tensor("ws", [NBLK * 128, 2 * NC_SHARD], F16,
                          kind="ExternalInput").ap()
    out_d = nc.dram_tensor("out", [M, NC_SHARD], F32, kind="ExternalOutput").ap()

    CHUNKS = [(c, min(512, NC_SHARD - c)) for c in range(0, NC_SHARD, 512)]

    with tile.TileContext(nc) as tc:
        ctx = contextlib.ExitStack()
        with ctx:
            consts = ctx.enter_context(tc.tile_pool(name="consts", bufs=1))
            wpool = ctx.enter_context(tc.tile_pool(name="w", bufs=1))
            wqld = ctx.enter_context(tc.tile_pool(name="wqld", bufs=2))
            wsb = ctx.enter_context(tc.tile_pool(name="ws", bufs=2))
            xp = ctx.enter_context(tc.tile_pool(name="x", bufs=3))
            up = ctx.enter_context(tc.tile_pool(name="u", bufs=3))
            atp = ctx.enter_context(tc.tile_pool(name="at", bufs=4))
            tre = ctx.enter_context(tc.tile_pool(name="tree", bufs=1))
            outp = ctx.enter_context(tc.tile_pool(name="o", bufs=1))
            vecs = ctx.enter_context(tc.tile_pool(name="v", bufs=8))
            ps_out = ctx.enter_context(
                tc.tile_pool(name="pso", bufs=2, space="PSUM"))
            ps_tr = ctx.enter_context(
                tc.tile_pool(name="pst", bufs=2, space="PSUM"))

            magic_vec = consts.tile([128, 1], F32)
            nc.vector.memset(magic_vec[:], MAGIC)
            ident = consts.tile([128, 128], F16)
            make_identity(nc, ident[:])

            # W holds all dequantized weights, k-major: [k%128, g, n]
            W = wpool.tile([128, NGRP * NC_SHARD], F16)

            x_of = {}
            scale_of = {}
            inv_of = {}
            a_of = {}
            at_of = {}

            def load_x(m, chunked=False):
                x_t = xp.tile([128, K], F16, tag="xt")
                if chunked:  # startup: stats can begin before full row lands
                    for j in range(4):
                        sl = slice(j * 1024, (j + 1) * 1024)
                        nc.scalar.dma_start(x_t[:, sl],
                                            x_d[m * 128:(m + 1) * 128, sl])
                else:
                    nc.scalar.dma_start(x_t[:], x_d[m * 128:(m + 1) * 128, :])
                x_of[m] = x_t

            def stats(m, chunked=False):
                """DVE row min/max via fp16 tensor_tensor tree + 1x reduce."""
                x_t = x_of[m]
                mx = vecs.tile([128, 1], F32, tag="mx")
                mn = vecs.tile([128, 1], F32, tag="mn")
                if chunked:  # per-1024 chunk partials, gated on each x DMA
                    mxp = vecs.tile([128, 4], F32, tag="mxp")
                    mnp = vecs.tile([128, 4], F32, tag="mnp")
                    for j in range(4):
                        lo = slice(j * 1024, j * 1024 + 512)
                        hi = slice(j * 1024 + 512, (j + 1) * 1024)
                        for (op, prt) in ((ALU.max, mxp), (ALU.min, mnp)):
                            s2 = tre.tile([128, 512], F16, tag="s2c")
                            nc.vector.tensor_tensor(s2[:], x_t[:, lo],
                                                    x_t[:, hi], op)
                            nc.vector.tensor_reduce(prt[:, j:j + 1], s2[:],
                                                    axis=AX.X, op=op)
                    nc.vector.tensor_reduce(mx[:], mxp[:], axis=AX.X, op=ALU.max)
                    nc.vector.tensor_reduce(mn[:], mnp[:], axis=AX.X, op=ALU.min)
                else:
                    for (op, dst) in ((ALU.max, mx), (ALU.min, mn)):
                        s1 = tre.tile([128, 2048], F16, tag="s1")
                        nc.vector.tensor_tensor(s1[:], x_t[:, :2048],
                                                x_t[:, 2048:], op)
                        s2 = tre.tile([128, 1024], F16, tag="s2")
                        nc.vector.tensor_tensor(s2[:], s1[:, :1024],
                                                s1[:, 1024:], op)
                        nc.vector.tensor_reduce(dst[:], s2[:], axis=AX.X, op=op)
                xc = vecs.tile([128, 1], F32, tag="xc")
                nc.vector.tensor_scalar(xc[:], mx[:], 0.0, None, ALU.max)
                nn_ = vecs.tile([128, 1], F32, tag="nn")
                nc.vector.tensor_scalar(nn_[:], mn[:], 0.0, None, ALU.min)
                df = vecs.tile([128, 1], F32, tag="df")
                nc.vector.tensor_tensor(df[:], xc[:], nn_[:], ALU.subtract)
                sc = vecs.tile([128, 1], F32, tag="sc")
                nc.vector.tensor_scalar(sc[:], df[:], 1.0 / 255.0, 1e-9,
                                        ALU.mult, ALU.max)
                inv = vecs.tile([128, 1], F32, tag="inv")
                nc.vector.reciprocal(inv[:], sc[:])
                scale_of[m] = sc
                inv_of[m] = inv

            def u_pass(m):
                """ACT: u = f16(x*inv + 1536) — the f16 convert rounds (RNE)."""
                u = up.tile([128, K], F16, tag="u")
                nc.scalar.activation(u[:], x_of[m][:], ACTF.Identity,
                                     bias=magic_vec[:], scale=inv_of[m][:])
                a_of[m] = u

            def a_pass(m):
                """DVE (2x), in place: a = u - 1536 (exact ints in fp16)."""
                u = a_of[m]
                nc.vector.tensor_scalar(u[:], u[:], -MAGIC, None, ALU.add)

            def quant(m):
                stats(m)
                u_pass(m)
                a_pass(m)

            def trT(m):
                """PE transposes via fp16 psum (8 groups per stage) + ACT copy."""
                a_t = a_of[m]
                aT = atp.tile([128, NGRP, 128], F16, tag="aT")
                for q in range(4):
                    st = ps_tr.tile([128, 1024], F16, tag="st")
                    for j in range(8):
                        g = q * 8 + j
                        nc.tensor.transpose(st[:, j * 128:(j + 1) * 128],
                                            a_t[:, g * 128:(g + 1) * 128],
                                            ident[:])
                    nc.scalar.copy(aT[:, q * 8:(q + 1) * 8, :]
                                   .rearrange("p g m -> p (g m)"), st[:])
                at_of[m] = aT

            def ws_block(b):
                ws_bc = wsb.tile([128, 2 * NC_SHARD], F16, tag="wsb")
                nc.gpsimd.dma_start(ws_bc[:], ws_d[b * 128:(b + 1) * 128, :])
                return ws_bc

            def wq_deq(g, ws_bc, j):
                wq_t = wqld.tile([128, NC_SHARD], F16, tag="wq")
                nc.sync.dma_start(wq_t[:], wq_d[g * 128:(g + 1) * 128, :])
                nc.vector.tensor_tensor(
                    W[:, g * NC_SHARD:(g + 1) * NC_SHARD], wq_t[:],
                    ws_bc[:, j * NC_SHARD:(j + 1) * NC_SHARD], ALU.mult)

            def mm_group(psum, aT, g):
                for (c0, cw) in CHUNKS:
                    nc.tensor.matmul(psum[:, c0:c0 + cw],
                                     lhsT=aT[:, g, :],
                                     rhs=W[:, g * NC_SHARD + c0:
                                           g * NC_SHARD + c0 + cw],
                                     start=(g == 0), stop=(g == NGRP - 1))

            def evict(m, psum):
                o_t = outp.tile([128, NC_SHARD], F32, tag="o")
                nc.scalar.activation(o_t[:], psum[:], ACTF.Identity,
                                     bias=0.0, scale=scale_of[m][:])
                nc.sync.dma_start(out_d[m * 128:(m + 1) * 128, :], o_t[:])

            # ---- emission ----
            load_x(0, chunked=True)
            load_x(1)
            load_x(2)
            stats(0, chunked=True)
            u_pass(0)
            stats(1)
            u_pass(1)

            # W stream (2 wq groups + dequant per pre-broadcast ws block) with
            # m0/m1 a-passes and m2..m4 quant interleaved; PE transposes for
            # m0/m1 before the fused ramp, m2/m3 injected mid-ramp.
            pe_ramp = []  # (after_block, fn)

            for b in range(NBLK):
                ws_bc = ws_block(b)
                for j in range(2):
                    wq_deq(2 * b + j, ws_bc, j)
                if b == 0:
                    a_pass(0)
                if b == 1:
                    a_pass(1)
                if b == 2:
                    load_x(3)
                    stats(2)
                    u_pass(2)
                if b == 4:
                    a_pass(2)
                if b == 6:
                    load_x(4)
                    stats(3)
                    u_pass(3)
                if b == 8:
                    a_pass(3)
                if b == 10:
                    load_x(5)
                    stats(4)
                    u_pass(4)
                if b == 12:
                    a_pass(4)

            trT(0)
            trT(1)
            ps0 = ps_out.tile([128, NC_SHARD], F32, tag="psum")
            ps1 = ps_out.tile([128, NC_SHARD], F32, tag="psum")
            for g in range(NGRP):
                mm_group(ps0, at_of[0], g)
                mm_group(ps1, at_of[1], g)
                if g == 9:
                    trT(2)
                if g == 14:
                    trT(3)
            evict(0, ps0)
            evict(1, ps1)

            for m in range(2, MTILES):
                if m + 3 < MTILES:
                    load_x(m + 3)
                    quant(m + 3)
                if m + 2 < MTILES:
                    trT(m + 2)
                psum = ps_out.tile([128, NC_SHARD], F32, tag="psum")
                for g in range(NGRP):
                    mm_group(psum, at_of[m], g)
                evict(m, psum)

    nc.compile()
    _CACHE["nc"] = nc
    return nc


def kernel(x, weight_qvals, weight_scales, group_size):
    global LAST_RESULTS
    _install_axon_ntff_hook()
    from concourse.bass_utils import run_bass_kernel_spmd

    x = np.asarray(x, dtype=np.float32)
    wq = np.asarray(weight_qvals)
    ws = np.asarray(weight_scales, dtype=np.float32)
    assert int(group_size) == GS
    assert x.shape == (M, K) and wq.shape == (N, K) and ws.shape == (N, NGRP)

    nc = _build()

    x16 = x.astype(np.float16)
    in_maps = []
    for c in range(NCORES):
        sl = slice(c * NC_SHARD, (c + 1) * NC_SHARD)
        wq_c = np.ascontiguousarray(wq[sl].T).astype(np.float16)
        # ws block b rows: concat(ws[:,2b], ws[:,2b+1]) broadcast on 128 rows
        ws_t = ws[sl].T.astype(np.float16)  # [32, 1376]
        ws_rows = ws_t.reshape(NBLK, 2 * NC_SHARD)
        ws_c = np.ascontiguousarray(
            np.broadcast_to(ws_rows[:, None, :], (NBLK, 128, 2 * NC_SHARD))
        ).reshape(NBLK * 128, 2 * NC_SHARD)
        in_maps.append({"x": x16, "wq": wq_c, "ws": ws_c})

    res = run_bass_kernel_spmd(nc, in_maps, core_ids=list(range(NCORES)))
    LAST_RESULTS = res
    out = np.concatenate([r["out"] for r in res.results], axis=1)
    return out


if __name__ == "__main__":
    rng = np.random.default_rng(0)
    xv = rng.standard_normal((M, K)).astype(np.float32)
    wqv = rng.integers(-4, 4, (N, K)).astype(np.int32)
    wsv = (rng.random((N, NGRP)).astype(np.float32) * 0.02 + 1e-4)
    o = kernel(xv, wqv, wsv, GS)
    print("out shape:", o.shape, "finite:", np.isfinite(o).all())


# revision 18
# speedup vs baseline: 1.6444x; 1.1339x over previous
"""Trainium2 Bass kernel for Chn8ActGrp3WgtQuantizedLinear.

Computes: out = fake_quant8_per_row(x) @ dequant(weight_qvals, weight_scales).T

  x:             (1024, 4096)  f32
  weight_qvals:  (11008, 4096) int32, 3-bit values in [-4, 3]
  weight_scales: (11008, 32)   f32, one scale per (out-channel, 128-group)
  out:           (1024, 11008) f32
  group_size:    128

Strategy (tensor parallel over 8 NeuronCores; N=11008 -> 1376/core):
  - host repack (layout/dtype only): x -> fp16; wq -> K-major fp16
    [4096, 1376] (3-bit values exact in fp16); ws -> fp16 pre-broadcast
    [16*128, 2752] (block b = groups 2b/2b+1 on 128 partitions).
  - device per core:
      * dequant W[k,n] = wq * ws_bc on DVE (fp16 2x mode), streamed per
        k-group as the wq/ws DMAs land.
      * activation fake-quant per 128-row m-tile: row min/max via a
        tensor_tensor halving tree (fp16 2x) + one 1x reduce on DVE;
        scale/inv; u = ACT(x*inv + 1536) -> f16 (the output convert
        rounds to integer, RNE, since ulp(1536)=1 for |v|<512); then
        in-place DVE u -= 1536 -> exact integer activations in fp16.
        The clip to [qmin-z, qmax-z] is dropped: without clipping the
        zero-point cancels algebraically (a = round(x/s)); round(x/s)
        escapes the clip range only by 1 lsb on knife-edge row extremes,
        perturbing a handful of elements by one quant step.
      * aT[k, m] via PE transposes (32 per m-tile) staged through fp16
        PSUM tiles (8 groups each) + ACT copies to SBUF.
      * matmul: psum[m=128, n=1376] += aT[:,g,:].T @ W over 32 k-groups
        (512-col psum-bank chunks); m0/m1 group-interleaved so the W
        DMA/dequant ramp is consumed at 2 m-tiles per group; quant for
        m2..m4 pipelined inside the ramp.
      * evict with per-row scale: out = psum * scale[m] (ACT), DMA out.
  - host concatenates the 8 (1024, 1376) shards.
"""

import sys
import types

import numpy as np

M, K, N, GS = 1024, 4096, 11008, 128
NCORES = 8
NC_SHARD = N // NCORES  # 1376
NGRP = K // GS  # 32
NBLK = NGRP // 2  # 16 k-group-pair blocks for the ws stream
MTILES = M // 128  # 8
MAGIC = 1536.0  # 1.5 * 2**10: f16 output convert rounds x*inv to int (RNE)

_CACHE = {}
LAST_RESULTS = None


def _install_axon_ntff_hook():
    """Register the NTFF profile hook if the container's antenv lacks it.

    Only needed for trace=True (BASS_TRACE=1); degrades silently."""
    try:
        if "antenv.axon_hooks" in sys.modules:
            return
        import antenv

        mod = types.ModuleType("antenv.axon_hooks")
        _state = {"hook": None}
        mod.set_axon_ntff_profile_hook = lambda h: _state.__setitem__("hook", h)
        mod.get_axon_ntff_profile_hook = lambda: _state["hook"]
        sys.modules["antenv.axon_hooks"] = mod
        antenv.axon_hooks = mod

        from trn_agent_boot.trn_boot import _ntff_profile_via_ctypes

        mod.set_axon_ntff_profile_hook(
            _ntff_profile_via_ctypes("/opt/axon/libaxon_pjrt.so")
        )
    except Exception:
        pass


def _build():
    if "nc" in _CACHE:
        return _CACHE["nc"]

    import contextlib

    import concourse.tile as tile
    from concourse import bacc, mybir
    from concourse.masks import make_identity

    dt = mybir.dt
    F32, F16 = dt.float32, dt.float16
    ALU = mybir.AluOpType
    ACTF = mybir.ActivationFunctionType
    AX = mybir.AxisListType

    nc = bacc.Bacc("TRN2", target_bir_lowering=False, debug=False,
                   num_devices=NCORES)

    x_d = nc.dram_tensor("x", [M, K], F16, kind="ExternalInput").ap()
    wq_d = nc.dram_tensor("wq", [K, NC_SHARD], F16, kind="ExternalInput").ap()
    ws_d = nc.dram_tensor("ws", [NBLK * 128, 2 * NC_SHARD], F16,
                          kind="ExternalInput").ap()
    out_d = nc.dram_tensor("out", [M, NC_SHARD], F32, kind="ExternalOutput").ap()

    CHUNKS = [(c, min(512, NC_SHARD - c)) for c in range(0, NC_SHARD, 512)]

    with tile.TileContext(nc) as tc:
        ctx = contextlib.ExitStack()
        with ctx:
            consts = ctx.enter_context(tc.tile_pool(name="consts", bufs=1))
            wpool = ctx.enter_context(tc.tile_pool(name="w", bufs=1))
            wqld = ctx.enter_context(tc.tile_pool(name="wqld", bufs=4))
            wsb = ctx.enter_context(tc.tile_pool(name="ws", bufs=3))
            xp = ctx.enter_context(tc.tile_pool(name="x", bufs=3))
            up = ctx.enter_context(tc.tile_pool(name="u", bufs=3))
            atp = ctx.enter_context(tc.tile_pool(name="at", bufs=4))
            tre = ctx.enter_context(tc.tile_pool(name="tree", bufs=1))
            outp = ctx.enter_context(tc.tile_pool(name="o", bufs=1))
            vecs = ctx.enter_context(tc.tile_pool(name="v", bufs=8))
            ps_out = ctx.enter_context(
                tc.tile_pool(name="pso", bufs=2, space="PSUM"))
            ps_tr = ctx.enter_context(
                tc.tile_pool(name="pst", bufs=2, space="PSUM"))

            magic_vec = consts.tile([128, 1], F32)
            nc.vector.memset(magic_vec[:], MAGIC)
            ident = consts.tile([128, 128], F16)
            make_identity(nc, ident[:])

            # W holds all dequantized weights, k-major: [k%128, g, n]
            W = wpool.tile([128, NGRP * NC_SHARD], F16)

            x_of = {}
            scale_of = {}
            inv_of = {}
            a_of = {}
            at_of = {}

            def load_x(m, chunked=False):
                x_t = xp.tile([128, K], F16, tag="xt")
                if chunked:  # startup: stats can begin before full row lands
                    for j in range(4):
                        sl = slice(j * 1024, (j + 1) * 1024)
                        nc.scalar.dma_start(x_t[:, sl],
                                            x_d[m * 128:(m + 1) * 128, sl])
                else:
                    nc.scalar.dma_start(x_t[:], x_d[m * 128:(m + 1) * 128, :])
                x_of[m] = x_t

            def stats(m, chunked=False):
                """DVE row min/max via fp16 tensor_tensor tree + 1x reduce."""
                x_t = x_of[m]
                mx = vecs.tile([128, 1], F32, tag="mx")
                mn = vecs.tile([128, 1], F32, tag="mn")
                if chunked:  # per-1024 chunk partials, gated on each x DMA
                    mxp = vecs.tile([128, 4], F32, tag="mxp")
                    mnp = vecs.tile([128, 4], F32, tag="mnp")
                    for j in range(4):
                        lo = slice(j * 1024, j * 1024 + 512)
                        hi = slice(j * 1024 + 512, (j + 1) * 1024)
                        for (op, prt) in ((ALU.max, mxp), (ALU.min, mnp)):
                            s2 = tre.tile([128, 512], F16, tag="s2c")
                            nc.vector.tensor_tensor(s2[:], x_t[:, lo],
                                                    x_t[:, hi], op)
                            nc.vector.tensor_reduce(prt[:, j:j + 1], s2[:],
                                                    axis=AX.X, op=op)
                    nc.vector.tensor_reduce(mx[:], mxp[:], axis=AX.X, op=ALU.max)
                    nc.vector.tensor_reduce(mn[:], mnp[:], axis=AX.X, op=ALU.min)
                else:
                    for (op, dst) in ((ALU.max, mx), (ALU.min, mn)):
                        s1 = tre.tile([128, 2048], F16, tag="s1")
                        nc.vector.tensor_tensor(s1[:], x_t[:, :2048],
                                                x_t[:, 2048:], op)
                        s2 = tre.tile([128, 1024], F16, tag="s2")
                        nc.vector.tensor_tensor(s2[:], s1[:, :1024],
                                                s1[:, 1024:], op)
                        nc.vector.tensor_reduce(dst[:], s2[:], axis=AX.X, op=op)
                xc = vecs.tile([128, 1], F32, tag="xc")
                nc.vector.tensor_scalar(xc[:], mx[:], 0.0, None, ALU.max)
                nn_ = vecs.tile([128, 1], F32, tag="nn")
                nc.vector.tensor_scalar(nn_[:], mn[:], 0.0, None, ALU.min)
                df = vecs.tile([128, 1], F32, tag="df")
                nc.vector.tensor_tensor(df[:], xc[:], nn_[:], ALU.subtract)
                sc = vecs.tile([128, 1], F32, tag="sc")
                nc.vector.tensor_scalar(sc[:], df[:], 1.0 / 255.0, 1e-9,
                                        ALU.mult, ALU.max)
                inv = vecs.tile([128, 1], F32, tag="inv")
                nc.vector.reciprocal(inv[:], sc[:])
                scale_of[m] = sc
                inv_of[m] = inv

            def u_pass(m):
                """ACT: u = f16(x*inv + 1536) — the f16 convert rounds (RNE)."""
                u = up.tile([128, K], F16, tag="u")
                nc.scalar.activation(u[:], x_of[m][:], ACTF.Identity,
                                     bias=magic_vec[:], scale=inv_of[m][:])
                a_of[m] = u

            def a_pass(m):
                """DVE (2x), in place: a = u - 1536 (exact ints in fp16)."""
                u = a_of[m]
                nc.vector.tensor_scalar(u[:], u[:], -MAGIC, None, ALU.add)

            def quant(m):
                stats(m)
                u_pass(m)
                a_pass(m)

            def trT(m):
                """PE transposes via fp16 psum (8 groups per stage) + ACT copy."""
                a_t = a_of[m]
                aT = atp.tile([128, NGRP, 128], F16, tag="aT")
                for q in range(4):
                    st = ps_tr.tile([128, 1024], F16, tag="st")
                    for j in range(8):
                        g = q * 8 + j
                        nc.tensor.transpose(st[:, j * 128:(j + 1) * 128],
                                            a_t[:, g * 128:(g + 1) * 128],
                                            ident[:])
                    nc.scalar.copy(aT[:, q * 8:(q + 1) * 8, :]
                                   .rearrange("p g m -> p (g m)"), st[:])
                at_of[m] = aT

            def ws_block(b):
                ws_bc = wsb.tile([128, 2 * NC_SHARD], F16, tag="wsb")
                nc.gpsimd.dma_start(ws_bc[:], ws_d[b * 128:(b + 1) * 128, :])
                return ws_bc

            def wq_deq(g, ws_bc, j):
                wq_t = wqld.tile([128, NC_SHARD], F16, tag="wq")
                nc.sync.dma_start(wq_t[:], wq_d[g * 128:(g + 1) * 128, :])
                nc.vector.tensor_tensor(
                    W[:, g * NC_SHARD:(g + 1) * NC_SHARD], wq_t[:],
                    ws_bc[:, j * NC_SHARD:(j + 1) * NC_SHARD], ALU.mult)

            def mm_group(psum, aT, g):
                for (c0, cw) in CHUNKS:
                    nc.tensor.matmul(psum[:, c0:c0 + cw],
                                     lhsT=aT[:, g, :],
                                     rhs=W[:, g * NC_SHARD + c0:
                                           g * NC_SHARD + c0 + cw],
                                     start=(g == 0), stop=(g == NGRP - 1))

            def evict(m, psum):
                o_t = outp.tile([128, NC_SHARD], F32, tag="o")
                nc.scalar.activation(o_t[:], psum[:], ACTF.Identity,
                                     bias=0.0, scale=scale_of[m][:])
                nc.sync.dma_start(out_d[m * 128:(m + 1) * 128, :], o_t[:])

            # ---- emission ----
            load_x(0)
            load_x(1)
            load_x(2)
            quant(0)
            quant(1)
            trT(0)
            trT(1)

            # Fused W stream + m0/m1 matmul ramp: each block's 2 groups are
            # dequantized and immediately consumed by both m-tiles, so the
            # ramp is paced by the wq/ws DMA streams. Quant for m2..m4 is
            # pipelined into the ramp on the DVE/ACT slack.
            ps0 = ps_out.tile([128, NC_SHARD], F32, tag="psum")
            ps1 = ps_out.tile([128, NC_SHARD], F32, tag="psum")
            for b in range(NBLK):
                ws_bc = ws_block(b)
                for j in range(2):
                    g = 2 * b + j
                    wq_deq(g, ws_bc, j)
                    mm_group(ps0, at_of[0], g)
                    mm_group(ps1, at_of[1], g)
                if b == 2:
                    load_x(3)
                    stats(2)
                    u_pass(2)
                if b == 4:
                    a_pass(2)
                if b == 5:
                    trT(2)
                if b == 6:
                    load_x(4)
                    stats(3)
                    u_pass(3)
                if b == 8:
                    a_pass(3)
                if b == 9:
                    trT(3)
                if b == 10:
                    load_x(5)
                    stats(4)
                    u_pass(4)
                if b == 12:
                    a_pass(4)
            evict(0, ps0)
            evict(1, ps1)

            for m in range(2, MTILES):
                if m + 3 < MTILES:
                    if m + 3 not in x_of:
                        load_x(m + 3)
                    quant(m + 3)
                if m + 2 < MTILES:
                    trT(m + 2)
                psum = ps_out.tile([128, NC_SHARD], F32, tag="psum")
                for g in range(NGRP):
                    mm_group(psum, at_of[m], g)
                evict(m, psum)

    nc.compile()
    _CACHE["nc"] = nc
    return nc


def kernel(x, weight_qvals, weight_scales, group_size):
    global LAST_RESULTS
    _install_axon_ntff_hook()
    from concourse.bass_utils import run_bass_kernel_spmd

    x = np.asarray(x, dtype=np.float32)
    wq = np.asarray(weight_qvals)
    ws = np.asarray(weight_scales, dtype=np.float32)
    assert int(group_size) == GS
    assert x.shape == (M, K) and wq.shape == (N, K) and ws.shape == (N, NGRP)

    nc = _build()

    x16 = x.astype(np.float16)
    in_maps = []
    for c in range(NCORES):
        sl = slice(c * NC_SHARD, (c + 1) * NC_SHARD)
        wq_c = np.ascontiguousarray(wq[sl].T).astype(np.float16)
        # ws block b rows: concat(ws[:,2b], ws[:,2b+1]) broadcast on 128 rows
        ws_t = ws[sl].T.astype(np.float16)  # [32, 1376]
        ws_rows = ws_t.reshape(NBLK, 2 * NC_SHARD)
        ws_c = np.ascontiguousarray(
            np.broadcast_to(ws_rows[:, None, :], (NBLK, 128, 2 * NC_SHARD))
        ).reshape(NBLK * 128, 2 * NC_SHARD)
        in_maps.append({"x": x16, "wq": wq_c, "ws": ws_c})

    res = run_bass_kernel_spmd(nc, in_maps, core_ids=list(range(NCORES)))
    LAST_RESULTS = res
    out = np.concatenate([r["out"] for r in res.results], axis=1)
    return out


if __name__ == "__main__":
    rng = np.random.default_rng(0)
    xv = rng.standard_normal((M, K)).astype(np.float32)
    wqv = rng.integers(-4, 4, (N, K)).astype(np.int32)
    wsv = (rng.random((N, NGRP)).astype(np.float32) * 0.02 + 1e-4)
    o = kernel(xv, wqv, wsv, GS)
    print("out shape:", o.shape, "finite:", np.isfinite(o).all())


# revision 23
# speedup vs baseline: 1.7070x; 1.0380x over previous
"""Trainium2 Bass kernel for Chn8ActGrp3WgtQuantizedLinear.

Computes: out = fake_quant8_per_row(x) @ dequant(weight_qvals, weight_scales).T

  x:             (1024, 4096)  f32
  weight_qvals:  (11008, 4096) int32, 3-bit values in [-4, 3]
  weight_scales: (11008, 32)   f32, one scale per (out-channel, 128-group)
  out:           (1024, 11008) f32
  group_size:    128

Strategy (tensor parallel over 8 NeuronCores; N=11008 -> 1376/core):
  - host repack (layout/dtype only): x -> fp16; wq -> K-major fp16
    [4096, 1376] (3-bit values exact in fp16); ws -> fp16 pre-broadcast
    [16*128, 2752] (block b = groups 2b/2b+1 on 128 partitions).
  - device per core:
      * dequant W[k,n] = wq * ws_bc on DVE (fp16 2x mode), streamed per
        k-group as the wq/ws DMAs land.
      * activation fake-quant per 128-row m-tile: row min/max via a
        tensor_tensor halving tree (fp16 2x) + one 1x reduce on DVE;
        scale/inv; u = ACT(x*inv + 1536) -> f16 (the output convert
        rounds to integer, RNE, since ulp(1536)=1 for |v|<512); then
        in-place DVE u -= 1536 -> exact integer activations in fp16.
        The clip to [qmin-z, qmax-z] is dropped: without clipping the
        zero-point cancels algebraically (a = round(x/s)); round(x/s)
        escapes the clip range only by 1 lsb on knife-edge row extremes,
        perturbing a handful of elements by one quant step.
      * aT[k, m] via PE transposes (32 per m-tile) staged through fp16
        PSUM tiles (8 groups each) + ACT copies to SBUF.
      * matmul: psum[m=128, n=1376] += aT[:,g,:].T @ W over 32 k-groups
        (512-col psum-bank chunks); m0/m1 group-interleaved so the W
        DMA/dequant ramp is consumed at 2 m-tiles per group; quant for
        m2..m4 pipelined inside the ramp.
      * evict with per-row scale: out = psum * scale[m] (ACT), DMA out.
  - host concatenates the 8 (1024, 1376) shards.
"""

import sys
import types

import ml_dtypes
import numpy as np

M, K, N, GS = 1024, 4096, 11008, 128
NCORES = 8
NC_SHARD = N // NCORES  # 1376
NGRP = K // GS  # 32
NBLK = NGRP // 2  # 16 k-group-pair blocks for the ws stream
MTILES = M // 128  # 8
MAGIC = 1536.0  # 1.5 * 2**10: f16 output convert rounds x*inv to int (RNE)

_CACHE = {}
LAST_RESULTS = None


def _install_axon_ntff_hook():
    """Register the NTFF profile hook if the container's antenv lacks it.

    Only needed for trace=True (BASS_TRACE=1); degrades silently."""
    try:
        if "antenv.axon_hooks" in sys.modules:
            return
        import antenv

        mod = types.ModuleType("antenv.axon_hooks")
        _state = {"hook": None}
        mod.set_axon_ntff_profile_hook = lambda h: _state.__setitem__("hook", h)
        mod.get_axon_ntff_profile_hook = lambda: _state["hook"]
        sys.modules["antenv.axon_hooks"] = mod
        antenv.axon_hooks = mod

        from trn_agent_boot.trn_boot import _ntff_profile_via_ctypes

        mod.set_axon_ntff_profile_hook(
            _ntff_profile_via_ctypes("/opt/axon/libaxon_pjrt.so")
        )
    except Exception:
        pass


def _build():
    if "nc" in _CACHE:
        return _CACHE["nc"]

    import contextlib

    import concourse.tile as tile
    from concourse import bacc, mybir
    from concourse.masks import make_identity

    dt = mybir.dt
    F32, F16 = dt.float32, dt.float16
    ALU = mybir.AluOpType
    ACTF = mybir.ActivationFunctionType
    AX = mybir.AxisListType

    nc = bacc.Bacc("TRN2", target_bir_lowering=False, debug=False,
                   num_devices=NCORES)

    x_d = nc.dram_tensor("x", [M, K], F16, kind="ExternalInput").ap()
    wq_d = nc.dram_tensor("wq", [K, NC_SHARD], dt.float8e4,
                          kind="ExternalInput").ap()
    ws_d = nc.dram_tensor("ws", [NBLK * 128, 2 * NC_SHARD], F16,
                          kind="ExternalInput").ap()
    out_d = nc.dram_tensor("out", [M, NC_SHARD], F32, kind="ExternalOutput").ap()

    CHUNKS = [(c, min(512, NC_SHARD - c)) for c in range(0, NC_SHARD, 512)]

    with tile.TileContext(nc) as tc:
        ctx = contextlib.ExitStack()
        with ctx:
            consts = ctx.enter_context(tc.tile_pool(name="consts", bufs=1))
            wpool = ctx.enter_context(tc.tile_pool(name="w", bufs=1))
            wqld = ctx.enter_context(tc.tile_pool(name="wqld", bufs=4))
            wsb = ctx.enter_context(tc.tile_pool(name="ws", bufs=3))
            xp = ctx.enter_context(tc.tile_pool(name="x", bufs=3))
            up = ctx.enter_context(tc.tile_pool(name="u", bufs=3))
            atp = ctx.enter_context(tc.tile_pool(name="at", bufs=4))
            tre = ctx.enter_context(tc.tile_pool(name="tree", bufs=1))
            outp = ctx.enter_context(tc.tile_pool(name="o", bufs=1))
            vecs = ctx.enter_context(tc.tile_pool(name="v", bufs=8))
            ps_out = ctx.enter_context(
                tc.tile_pool(name="pso", bufs=2, space="PSUM"))
            ps_tr = ctx.enter_context(
                tc.tile_pool(name="pst", bufs=2, space="PSUM"))

            magic_vec = consts.tile([128, 1], F32)
            nc.vector.memset(magic_vec[:], MAGIC)
            ident = consts.tile([128, 128], F16)
            make_identity(nc, ident[:])

            # W holds all dequantized weights, k-major: [k%128, g, n]
            W = wpool.tile([128, NGRP * NC_SHARD], F16)

            x_of = {}
            scale_of = {}
            inv_of = {}
            a_of = {}
            at_of = {}

            def load_x(m, chunked=False):
                x_t = xp.tile([128, K], F16, tag="xt")
                if chunked:  # startup: stats can begin before full row lands
                    for j in range(4):
                        sl = slice(j * 1024, (j + 1) * 1024)
                        nc.scalar.dma_start(x_t[:, sl],
                                            x_d[m * 128:(m + 1) * 128, sl])
                else:
                    nc.scalar.dma_start(x_t[:], x_d[m * 128:(m + 1) * 128, :])
                x_of[m] = x_t

            def stats(m, chunked=False):
                """DVE row min/max via fp16 tensor_tensor tree + 1x reduce."""
                x_t = x_of[m]
                mx = vecs.tile([128, 1], F32, tag="mx")
                mn = vecs.tile([128, 1], F32, tag="mn")
                if chunked:  # per-1024 chunk partials, gated on each x DMA
                    mxp = vecs.tile([128, 4], F32, tag="mxp")
                    mnp = vecs.tile([128, 4], F32, tag="mnp")
                    for j in range(4):
                        lo = slice(j * 1024, j * 1024 + 512)
                        hi = slice(j * 1024 + 512, (j + 1) * 1024)
                        for (op, prt) in ((ALU.max, mxp), (ALU.min, mnp)):
                            s2 = tre.tile([128, 512], F16, tag="s2c")
                            nc.vector.tensor_tensor(s2[:], x_t[:, lo],
                                                    x_t[:, hi], op)
                            nc.vector.tensor_reduce(prt[:, j:j + 1], s2[:],
                                                    axis=AX.X, op=op)
                    nc.vector.tensor_reduce(mx[:], mxp[:], axis=AX.X, op=ALU.max)
                    nc.vector.tensor_reduce(mn[:], mnp[:], axis=AX.X, op=ALU.min)
                else:
                    for (op, dst) in ((ALU.max, mx), (ALU.min, mn)):
                        s1 = tre.tile([128, 2048], F16, tag="s1")
                        nc.vector.tensor_tensor(s1[:], x_t[:, :2048],
                                                x_t[:, 2048:], op)
                        s2 = tre.tile([128, 1024], F16, tag="s2")
                        nc.vector.tensor_tensor(s2[:], s1[:, :1024],
                                                s1[:, 1024:], op)
                        nc.vector.tensor_reduce(dst[:], s2[:], axis=AX.X, op=op)
                xc = vecs.tile([128, 1], F32, tag="xc")
                nc.vector.tensor_scalar(xc[:], mx[:], 0.0, None, ALU.max)
                nn_ = vecs.tile([128, 1], F32, tag="nn")
                nc.vector.tensor_scalar(nn_[:], mn[:], 0.0, None, ALU.min)
                df = vecs.tile([128, 1], F32, tag="df")
                nc.vector.tensor_tensor(df[:], xc[:], nn_[:], ALU.subtract)
                sc = vecs.tile([128, 1], F32, tag="sc")
                nc.vector.tensor_scalar(sc[:], df[:], 1.0 / 255.0, 1e-9,
                                        ALU.mult, ALU.max)
                inv = vecs.tile([128, 1], F32, tag="inv")
                nc.vector.reciprocal(inv[:], sc[:])
                scale_of[m] = sc
                inv_of[m] = inv

            def u_pass(m):
                """ACT: u = f16(x*inv + 1536) — the f16 convert rounds (RNE)."""
                u = up.tile([128, K], F16, tag="u")
                nc.scalar.activation(u[:], x_of[m][:], ACTF.Identity,
                                     bias=magic_vec[:], scale=inv_of[m][:])
                a_of[m] = u

            def a_pass(m):
                """DVE (2x), in place: a = u - 1536 (exact ints in fp16)."""
                u = a_of[m]
                nc.vector.tensor_scalar(u[:], u[:], -MAGIC, None, ALU.add)

            def quant(m):
                stats(m)
                u_pass(m)
                a_pass(m)

            def trT(m):
                """PE transposes via fp16 psum (8 groups per stage) + ACT copy."""
                a_t = a_of[m]
                aT = atp.tile([128, NGRP, 128], F16, tag="aT")
                for q in range(4):
                    st = ps_tr.tile([128, 1024], F16, tag="st")
                    for j in range(8):
                        g = q * 8 + j
                        nc.tensor.transpose(st[:, j * 128:(j + 1) * 128],
                                            a_t[:, g * 128:(g + 1) * 128],
                                            ident[:])
                    nc.scalar.copy(aT[:, q * 8:(q + 1) * 8, :]
                                   .rearrange("p g m -> p (g m)"), st[:])
                at_of[m] = aT

            def ws_block(b):
                ws_bc = wsb.tile([128, 2 * NC_SHARD], F16, tag="wsb")
                nc.gpsimd.dma_start(ws_bc[:], ws_d[b * 128:(b + 1) * 128, :])
                return ws_bc

            def wq_deq(g, ws_bc, j):
                wq_t = wqld.tile([128, NC_SHARD], dt.float8e4, tag="wq")
                nc.sync.dma_start(wq_t[:], wq_d[g * 128:(g + 1) * 128, :])
                nc.vector.tensor_tensor(
                    W[:, g * NC_SHARD:(g + 1) * NC_SHARD], wq_t[:],
                    ws_bc[:, j * NC_SHARD:(j + 1) * NC_SHARD], ALU.mult)

            def mm_group(psum, aT, g):
                for (c0, cw) in CHUNKS:
                    nc.tensor.matmul(psum[:, c0:c0 + cw],
                                     lhsT=aT[:, g, :],
                                     rhs=W[:, g * NC_SHARD + c0:
                                           g * NC_SHARD + c0 + cw],
                                     start=(g == 0), stop=(g == NGRP - 1))

            def evict(m, psum):
                o_t = outp.tile([128, NC_SHARD], F32, tag="o")
                nc.scalar.activation(o_t[:], psum[:], ACTF.Identity,
                                     bias=0.0, scale=scale_of[m][:])
                nc.sync.dma_start(out_d[m * 128:(m + 1) * 128, :], o_t[:])

            # ---- emission ----
            load_x(0)
            load_x(1)
            load_x(2)
            quant(0)
            quant(1)
            trT(0)
            trT(1)

            # Fused W stream + m0/m1 matmul ramp: each block's 2 groups are
            # dequantized and immediately consumed by both m-tiles, so the
            # ramp is paced by the wq/ws DMA streams. Quant for m2..m4 is
            # pipelined into the ramp on the DVE/ACT slack.
            ps0 = ps_out.tile([128, NC_SHARD], F32, tag="psum")
            ps1 = ps_out.tile([128, NC_SHARD], F32, tag="psum")
            for b in range(NBLK):
                ws_bc = ws_block(b)
                for j in range(2):
                    g = 2 * b + j
                    wq_deq(g, ws_bc, j)
                    mm_group(ps0, at_of[0], g)
                    mm_group(ps1, at_of[1], g)
                if b == 5:
                    load_x(3)
                if b == 7:
                    quant(2)
                if b == 10:
                    load_x(4)
                if b == 12:
                    trT(2)
            quant(3)
            evict(0, ps0)
            evict(1, ps1)

            for m in range(2, MTILES):
                if m + 3 < MTILES and m + 3 not in x_of:
                    load_x(m + 3)
                if m + 2 < MTILES:
                    quant(m + 2)
                if m + 1 < MTILES:
                    trT(m + 1)
                psum = ps_out.tile([128, NC_SHARD], F32, tag="psum")
                for g in range(NGRP):
                    mm_group(psum, at_of[m], g)
                evict(m, psum)

    nc.compile()
    _CACHE["nc"] = nc
    return nc


def kernel(x, weight_qvals, weight_scales, group_size):
    global LAST_RESULTS
    _install_axon_ntff_hook()
    from concourse.bass_utils import run_bass_kernel_spmd

    x = np.asarray(x, dtype=np.float32)
    wq = np.asarray(weight_qvals)
    ws = np.asarray(weight_scales, dtype=np.float32)
    assert int(group_size) == GS
    assert x.shape == (M, K) and wq.shape == (N, K) and ws.shape == (N, NGRP)

    nc = _build()

    x16 = x.astype(np.float16)
    in_maps = []
    for c in range(NCORES):
        sl = slice(c * NC_SHARD, (c + 1) * NC_SHARD)
        wq_c = np.ascontiguousarray(wq[sl].T).astype(ml_dtypes.float8_e4m3fn)
        # ws block b rows: concat(ws[:,2b], ws[:,2b+1]) broadcast on 128 rows
        ws_t = ws[sl].T.astype(np.float16)  # [32, 1376]
        ws_rows = ws_t.reshape(NBLK, 2 * NC_SHARD)
        ws_c = np.ascontiguousarray(
            np.broadcast_to(ws_rows[:, None, :], (NBLK, 128, 2 * NC_SHARD))
        ).reshape(NBLK * 128, 2 * NC_SHARD)
        in_maps.append({"x": x16, "wq": wq_c, "ws": ws_c})

    res = run_bass_kernel_spmd(nc, in_maps, core_ids=list(range(NCORES)))
    LAST_RESULTS = res
    out = np.concatenate([r["out"] for r in res.results], axis=1)
    return out


if __name__ == "__main__":
    rng = np.random.default_rng(0)
    xv = rng.standard_normal((M, K)).astype(np.float32)
    wqv = rng.integers(-4, 4, (N, K)).astype(np.int32)
    wsv = (rng.random((N, NGRP)).astype(np.float32) * 0.02 + 1e-4)
    o = kernel(xv, wqv, wsv, GS)
    print("out shape:", o.shape, "finite:", np.isfinite(o).all())
